# revision 64
# baseline (speedup 1.0000x reference)
"""Trainium2 Bass kernel for nn_Actor (dense MLP trunk + 64 softmax heads).

Data-parallel over 8 NeuronCores: batch 4096 -> 512 rows/core, weights
replicated. Feature-major trunk (activations [features, batch]) so layer
outputs feed the next contraction without transposes; heads run batch-major
so per-head softmax reduces along the free dim.

Precision: trunk AND heads run fp8-e4m3 DoubleRow matmuls (256-deep
contraction per instruction; weights pre-scaled x256, x pre-scaled x32,
h1 stored x16, h2 stored x64 -- compensated via activation scale=).
The head bias rides a zero-padded 5th DR k-pair (constant-16 row in h2
against 1024*bh rows in wh), so no extra bias instruction exists at all.
Head columns are grouped per chunk [rsu segs | lay segs] so the softmax
reduces/muls are flat 3D DVE ops; output is stored bf16 in grouped order
and the host unpermutes + upcasts.

DMA: sync queue carries only w1 (half-tile ramp loads, then m-tile pairs
with 4KB runs) and later the output stores; gpsimd SWDGE carries xt in
ramp-matched chunks, then w2/wh gated behind the ramp matmuls; the scalar
queue only carries the tiny bias vectors so the ACT engine stays free.
"""

import os
import numpy as np

B, IN_DIM, HIDDEN, H2 = 4096, 2048, 2048, 1024
V, R, L = 16, 32, 8
OUTC = V * (2 * R + 2 * L)          # 1280
NCORES = 8
BC = B // NCORES                    # 512 batch rows per core
KT1 = IN_DIM // 128                 # 16 k-tiles, layer 1
MT1 = HIDDEN // 128                 # 16 m-tiles, layer 1
KT2 = HIDDEN // 128                 # 16 k-tiles, layer 2
MT2 = H2 // 128                     # 8 m-tiles, layer 2
KTH = H2 // 128                     # 8 k-tiles, heads
BT = BC // 128                      # 4 batch tiles per core
CW = 320                            # head chunk width = 4 vehicles
NCH = OUTC // CW                    # 4 chunks
VC = CW // (2 * (R + L))            # 4 vehicles per chunk

_CACHE = {}
LAST_RESULTS = None                 # BassKernelResults from the last run


def _colperm():
    # device column g -> original output column. Within each 320-col chunk
    # (4 vehicles) the columns are grouped [8 rsu segs x R | 8 lay segs x L]
    # so the on-device softmax sees uniform 3D patterns.
    p = np.empty(OUTC, dtype=np.int64)
    g = 0
    for c in range(NCH):
        for s in range(2 * VC):
            v, h = 4 * c + s // 2, s % 2
            for j in range(R):
                p[g] = v * 2 * (R + L) + h * R + j
                g += 1
        for s in range(2 * VC):
            v, h = 4 * c + s // 2, s % 2
            for j in range(L):
                p[g] = v * 2 * (R + L) + 2 * R + h * L + j
                g += 1
    assert g == OUTC
    return p


_COLPERM = _colperm()


def _build():
    import bass_rust
    import concourse.bacc as bacc
    import concourse.mybir as mybir
    import concourse.tile as tile

    F32 = mybir.dt.float32
    BF16 = mybir.dt.bfloat16
    F8 = mybir.dt.float8e4
    DR = mybir.MatmulPerfMode.DoubleRow
    Relu = mybir.ActivationFunctionType.Relu
    Exp = mybir.ActivationFunctionType.Exp
    X = mybir.AxisListType.X

    nc = bacc.Bacc("TRN2", target_bir_lowering=False, debug=False,
                   num_devices=NCORES)

    xt = nc.dram_tensor("xt", [128, KT1, BC], F8, kind="ExternalInput")
    w1t = nc.dram_tensor("w1t", [128, MT1, KT1, 128], F8, kind="ExternalInput")
    b1c = nc.dram_tensor("b1c", [128, MT1], F32, kind="ExternalInput")
    w2t = nc.dram_tensor("w2t", [128, MT2, KT2, 128], F8, kind="ExternalInput")
    b2c = nc.dram_tensor("b2c", [128, MT2], F32, kind="ExternalInput")
    # kp = KTH//2 is a bias pad: row (p=0, sub=0) of h2 holds a constant 16,
    # the matching wht rows hold 1024*bh, everything else zero. Columns are
    # grouped [all rsu-head segments | all lay-head segments] so the softmax
    # reductions see uniform 3D access patterns; the host unpermutes.
    wht = nc.dram_tensor("wht", [128, KTH // 2 + 1, 2, OUTC], F8,
                         kind="ExternalInput")
    out = nc.dram_tensor("out", [BC, OUTC], BF16, kind="ExternalOutput")

    with tile.TileContext(nc) as tc:
        with (
            tc.tile_pool(name="const", bufs=1) as cp,
            tc.tile_pool(name="wpool", bufs=8) as wp,
            tc.tile_pool(name="sm", bufs=4) as sp,
            tc.tile_pool(name="ps", bufs=4, space="PSUM") as ps,
            tc.tile_pool(name="psh", bufs=4, space="PSUM") as psh,
        ):
            # PE warmup: dummy matmuls bridge the gap until the first w1/xt
            # tiles land so the HAM clock ramp overlaps the DMA-bound start.
            warm = cp.tile([128, 128], BF16, tag="warm")
            nc.gpsimd.memset(warm[:], 0.0)
            wacc = psh.tile([128, CW], F32, tag="hacc")
            for i in range(20):
                nc.tensor.matmul(wacc[:, 0:128], warm[:], warm[:],
                                 start=True, stop=True)

            xt_sb = cp.tile([128, KT1, BC], F8, tag="xt")
            h1_sb = cp.tile([128, MT1, BC], F8, tag="h1")
            # pre-tiled for DoubleRow: [kpair][bt|chunk][2][dim] slices are
            # contiguous [128, 2, d] blocks
            h2_sb = cp.tile([128, KTH // 2 + 1, BT, 2, 128], F8, tag="h2")
            wh_sb = cp.tile([128, KTH // 2 + 1, 2, OUTC], F8, tag="wh")
            w2_sb = cp.tile([128, MT2, KT2, 128], F8, tag="w2")
            b1_sb = cp.tile([128, MT1], F32, tag="b1")
            b2_sb = cp.tile([128, MT2], F32, tag="b2")
            # bias pad k-tiles of h2: all zero except partition-0/sub-0 = 16
            nc.vector.memset(h2_sb[:, KTH // 2], 0.0)
            nc.vector.memset(h2_sb[0:1, KTH // 2, :, 0, :], 16.0)

            # scalar queue, strict FIFO and few issues: b1, xt (small first
            # chunk so the ramp can start), b2, bh. w2/wh go on the gpsimd
            # SWDGE queue, gated behind L1 progress so their transfers can't
            # steal HBM bandwidth from the w1/xt streams at the start.
            # xt rides the gpsimd SWDGE queue in fine chunks matched to the
            # ramp's k-pair consumption; w2/wh follow on the same queue,
            # gated behind the ramp matmuls. The scalar queue only carries
            # the tiny bias vectors, keeping the ACT engine free.
            for k0, kn in [(0, 2), (2, 2), (4, 4), (8, 4), (12, 4)]:
                nc.gpsimd.dma_start(xt_sb[:, k0:k0 + kn, :],
                                    xt.ap()[:, k0:k0 + kn, :])
            nc.scalar.dma_start(b1_sb[:], b1c.ap())
            nc.scalar.dma_start(b2_sb[:], b2c.ap())

            # sync queue: w1 only. Ramp tiles m0..3 arrive as half tiles so
            # the k-outer ramp is fed at finer granularity; the rest are
            # m-tile pairs with 4KB descriptor runs.
            RM = 4                       # m-tiles in the k-outer ramp group
            rw1 = []
            for mi in range(RM):
                w1m = wp.tile([128, KT1, 128], F8, tag="w1m")
                rw1.append(w1m)
            for k0 in range(0, KT1, 8):
                for mi in range(RM):
                    nc.sync.dma_start(rw1[mi][:, k0:k0 + 8, :],
                                      w1t.ap()[:, mi, k0:k0 + 8, :])

            raccs = []
            for mi in range(RM):
                racc = ps.tile([128, 512], F32, tag="acc")
                raccs.append(racc)
            ramp_mm = None
            for k in range(0, KT1, 2):
                for mi in range(RM):
                    ramp_mm = nc.tensor.matmul(raccs[mi][:],
                                               rw1[mi][:, k:k + 2, :],
                                               xt_sb[:, k:k + 2, :],
                                               start=(k == 0),
                                               stop=(k == KT1 - 2),
                                               perf_mode=DR)
            for mi in range(RM):
                nc.scalar.activation(h1_sb[:, mi, :], raccs[mi][:],
                                     Relu, bias=b1_sb[:, mi:mi + 1],
                                     scale=1.0 / 512.0)

            # w2/wh prefetch on gpsimd, paced behind the ramp matmuls (by
            # then xt is fully resident and the w1 stream has headroom)
            for m0 in range(0, MT2, 4):
                w2d = nc.gpsimd.dma_start(w2_sb[:, m0:m0 + 4],
                                          w2t.ap()[:, m0:m0 + 4])
                bass_rust.add_dep_helper(w2d.ins, ramp_mm.ins, sync=True,
                                         reason="pace w2 prefetch")
            whd = nc.gpsimd.dma_start(wh_sb[:], wht.ap())
            bass_rust.add_dep_helper(whd.ins, ramp_mm.ins, sync=True,
                                     reason="pace wh prefetch")

            # Layer 1 remainder, m-tile pairs: h1[m] = relu(w1[:,m].T @ xt + b1)
            for m0 in range(RM, MT1, 2):
                w1m2 = wp.tile([128, 2, KT1, 128], F8, tag="w1m2")
                nc.sync.dma_start(w1m2[:], w1t.ap()[:, m0:m0 + 2])
                for j in range(2):
                    m = m0 + j
                    acc = ps.tile([128, 512], F32, tag="acc")
                    for k in range(0, KT1, 2):
                        nc.tensor.matmul(acc[:], w1m2[:, j, k:k + 2, :],
                                         xt_sb[:, k:k + 2, :],
                                         start=(k == 0), stop=(k == KT1 - 2),
                                         perf_mode=DR)
                    nc.scalar.activation(h1_sb[:, m, :], acc[:], Relu,
                                         bias=b1_sb[:, m:m + 1],
                                         scale=1.0 / 512.0)

            # Layer 2: h2[m] = relu(sum_k w2[k,m].T @ h1[k] + b2[m]),
            # stored x64 in fp8 for the DoubleRow head matmuls.
            for m in range(MT2):
                acc = ps.tile([128, 512], F32, tag="acc")
                for k in range(0, KT2, 2):
                    nc.tensor.matmul(acc[:], w2_sb[:, m, k:k + 2, :],
                                     h1_sb[:, k:k + 2, :],
                                     start=(k == 0), stop=(k == KT2 - 2),
                                     perf_mode=DR)
                nc.scalar.activation(h2_sb[:, m // 2, :, m % 2, :], acc[:],
                                     Relu, bias=b2_sb[:, m:m + 1],
                                     scale=1.0 / 64.0)

            # Heads: acc[b, c] = 16384*(h2[:, b].T @ wh[:, c] + bh[c]),
            # fp8 DoubleRow + a ones-row bf16 matmul for the bias. Softmax:
            # exp on ACT (scale folds the x16384 away), f32 reduces + recip
            # on DVE, divisor muls on GpSimd (split with DVE for the last
            # pair to shorten the drain), bf16 stores on the idle sync queue.
            # Per-chunk pipeline with a 4-deep PSUM ring: matmuls for chunk
            # c+3 can run while chunks c..c+2 drain through exp/reduce/mul.
            # Reciprocal is batched per chunk-pair to bound DVE overheads.
            for bt in range(BT):
                bsl = slice(bt * 128, (bt + 1) * 128)
                for c in range(NCH):
                    acc = psh.tile([128, CW], F32, tag="hacc")
                    for kp in range(KTH // 2 + 1):
                        nc.tensor.matmul(acc[:], h2_sb[:, kp, bt],
                                         wh_sb[:, kp, :,
                                               c * CW:(c + 1) * CW],
                                         start=(kp == 0),
                                         stop=(kp == KTH // 2),
                                         perf_mode=DR)

                    if c % 2 == 0:
                        et = sp.tile([128, 2 * CW], F32, tag="et")
                        sums = sp.tile([128, 8 * VC], F32, tag="sums")
                        rec = sp.tile([128, 8 * VC], F32, tag="rec")
                        o_sb = sp.tile([128, 2 * CW], BF16, tag="o")
                    half = (c % 2) * CW
                    nc.scalar.activation(et[:, half:half + CW], acc[:], Exp,
                                         scale=1.0 / 16384.0)

                    # grouped columns: chunk = [2*VC rsu segs x R | 2*VC lay
                    # segs x L] -- flat 3D views for the DVE
                    NRC = 2 * VC * R
                    rsu3 = et[:, half:half + NRC].rearrange(
                        "p (s c) -> p s c", s=2 * VC)
                    lay3 = et[:, half + NRC:half + CW].rearrange(
                        "p (s c) -> p s c", s=2 * VC)
                    sh = (c % 2) * 4 * VC
                    nc.vector.reduce_sum(
                        out=sums[:, sh:sh + 2 * VC].unsqueeze(2),
                        in_=rsu3, axis=X)
                    nc.vector.reduce_sum(
                        out=sums[:, sh + 2 * VC:sh + 4 * VC].unsqueeze(2),
                        in_=lay3, axis=X)
                    if c % 2 == 1:
                        nc.vector.reciprocal(rec[:, 0:8 * VC],
                                             sums[:, 0:8 * VC])
                    last = (bt == BT - 1 and c == NCH - 1)

                    # divisor muls trail one chunk behind the recip
                    if c % 2 == 1:
                        for cc in (c - 1, c):
                            h2_ = (cc % 2) * CW
                            sh2 = (cc % 2) * 4 * VC
                            orsu = o_sb[:, h2_:h2_ + NRC].rearrange(
                                "p (s c) -> p s c", s=2 * VC)
                            olay = o_sb[:, h2_ + NRC:h2_ + CW].rearrange(
                                "p (s c) -> p s c", s=2 * VC)
                            ersu = et[:, h2_:h2_ + NRC].rearrange(
                                "p (s c) -> p s c", s=2 * VC)
                            elay = et[:, h2_ + NRC:h2_ + CW].rearrange(
                                "p (s c) -> p s c", s=2 * VC)
                            r_r = rec[:, sh2:sh2 + 2 * VC]
                            r_l = rec[:, sh2 + 2 * VC:sh2 + 4 * VC]
                            # last bt: lay-mul rides vector concurrently so
                            # the drain's mul stage is just the rsu-mul
                            leng = nc.vector if bt == BT - 1 else nc.gpsimd
                            nc.gpsimd.tensor_mul(
                                orsu, ersu,
                                r_r.unsqueeze(2).broadcast_to(
                                    [128, 2 * VC, R]))
                            leng.tensor_mul(
                                olay, elay,
                                r_l.unsqueeze(2).broadcast_to(
                                    [128, 2 * VC, L]))
                            if last:
                                nc.sync.dma_start(
                                    out.ap()[bsl, cc * CW:(cc + 1) * CW],
                                    o_sb[:, h2_:h2_ + CW])
                        if not last:
                            nc.sync.dma_start(
                                out.ap()[bsl, (c - 1) * CW:(c + 1) * CW],
                                o_sb[:, 0:2 * CW])

    nc.compile()
    return nc


def _prep_shared(w1, b1, w2, b2, w_rsu, b_rsu, w_lay, b_lay):
    import ml_dtypes
    f, bf = np.float32, ml_dtypes.bfloat16
    f8 = ml_dtypes.float8_e4m3
    # partition-major [128, MT1, KT1, 128]: paired-tile DMA slices match the
    # SBUF destination dim order and give 4KB-contiguous runs per partition
    w1t = np.ascontiguousarray(
        np.clip(w1 * 256.0, -240, 240).astype(f8)
        .reshape(KT1, 128, MT1, 128).transpose(1, 2, 0, 3))
    # partition-major [128, MT2, KT2, 128] so paired-tile DMA slices match
    # the SBUF destination's dimension order
    w2t = np.ascontiguousarray(
        np.clip(w2 * 256.0, -240, 240).astype(f8)
        .reshape(KT2, 128, MT2, 128).transpose(1, 2, 0, 3))
    b1c = np.ascontiguousarray(16.0 * b1.reshape(MT1, 128).T, dtype=f)
    b2c = np.ascontiguousarray(64.0 * b2.reshape(MT2, 128).T, dtype=f)

    wh = np.empty((H2, OUTC), dtype=f)
    bh = np.empty((OUTC,), dtype=f)
    for v in range(V):
        c = 2 * (R + L) * v
        wh[:, c:c + R] = w_rsu[2 * v]
        wh[:, c + R:c + 2 * R] = w_rsu[2 * v + 1]
        wh[:, c + 2 * R:c + 2 * R + L] = w_lay[2 * v]
        wh[:, c + 2 * R + L:c + 2 * (R + L)] = w_lay[2 * v + 1]
        bh[c:c + R] = b_rsu[2 * v]
        bh[c + R:c + 2 * R] = b_rsu[2 * v + 1]
        bh[c + 2 * R:c + 2 * R + L] = b_lay[2 * v]
        bh[c + 2 * R + L:c + 2 * (R + L)] = b_lay[2 * v + 1]
    # group columns [all rsu-head segments | all lay-head segments] so the
    # on-device softmax sees uniform 3D patterns; _COLPERM maps back
    whg = wh[:, _COLPERM]
    bhg = bh[_COLPERM]
    # [128p, kpair, sub, col]; the extra kpair is the bias pad: row
    # (p=0, sub=0) carries 1024*bh, pairing with h2's constant-16 row to
    # add 16384*bh to the logits.
    whp = np.clip(whg * 256.0, -240, 240).astype(f8) \
        .reshape(KTH // 2, 2, 128, OUTC).transpose(2, 0, 1, 3)
    wht = np.zeros((128, KTH // 2 + 1, 2, OUTC), dtype=f8)
    wht[:, :KTH // 2] = whp
    wht[0, KTH // 2, 0, :] = np.clip(1024.0 * bhg, -240, 240).astype(f8)
    return {"w1t": w1t, "b1c": b1c, "w2t": w2t, "b2c": b2c, "wht": wht}


def kernel(x, w1, b1, w2, b2, w_rsu, b_rsu, w_lay, b_lay):
    global LAST_RESULTS
    import ml_dtypes
    from concourse.bass_utils import run_bass_kernel_spmd

    if "nc" not in _CACHE:
        _CACHE["nc"] = _build()
    nc = _CACHE["nc"]

    shared = _prep_shared(np.asarray(w1, np.float32), np.asarray(b1, np.float32),
                          np.asarray(w2, np.float32), np.asarray(b2, np.float32),
                          np.asarray(w_rsu, np.float32), np.asarray(b_rsu, np.float32),
                          np.asarray(w_lay, np.float32), np.asarray(b_lay, np.float32))

    # x [B, IN] -> per-core xt [128, KT1, BC] with [p, k, n] = x[core*BC+n, k*128+p]
    # fp8 e4m3 with x*32 so small values clear the subnormal range; the
    # combined 32*256 scale comes out in the L1 relu (scale=1/512 with h1 x16)
    xt_full = np.clip(np.ascontiguousarray(np.asarray(x, np.float32).T) * 32.0,
                      -240, 240) \
        .astype(ml_dtypes.float8_e4m3).reshape(KT1, 128, B).transpose(1, 0, 2)
    in_maps = []
    for c in range(NCORES):
        m = dict(shared)
        m["xt"] = np.ascontiguousarray(xt_full[:, :, c * BC:(c + 1) * BC])
        in_maps.append(m)

    trace = os.environ.get("KERNEL_TRACE", "") == "1"
    LAST_RESULTS = run_bass_kernel_spmd(nc, in_maps, core_ids=list(range(NCORES)),
                                        trace=trace)
    dev = np.concatenate(
        [r["out"].astype(np.float32) for r in LAST_RESULTS.results], axis=0)
    full = np.empty_like(dev)
    full[:, _COLPERM] = dev          # undo the grouped column order
    return full


# revision 65
# speedup vs baseline: 1.1883x; 1.1883x over previous
"""Trainium2 Bass kernel for nn_Actor (dense MLP trunk + 64 softmax heads).

Data-parallel over 8 NeuronCores: batch 4096 -> 512 rows/core, weights
replicated. Feature-major trunk (activations [features, batch]) so layer
outputs feed the next contraction without transposes; heads run batch-major
so per-head softmax reduces along the free dim.

Precision: trunk AND heads run fp8-e4m3 DoubleRow matmuls (256-deep
contraction per instruction; weights pre-scaled x256, x pre-scaled x32,
h1 stored x16, h2 stored x64 -- compensated via activation scale=).
The head bias rides a zero-padded 5th DR k-pair (constant-16 row in h2
against 1024*bh rows in wh), so no extra bias instruction exists at all.
Head columns are grouped per chunk [rsu segs | lay segs] so the softmax
reduces/muls are flat 3D DVE ops; output is stored bf16 in grouped order
and the host unpermutes + upcasts.

DMA: sync queue carries only w1 (half-tile ramp loads, then m-tile pairs
with 4KB runs) and later the output stores; gpsimd SWDGE carries xt in
ramp-matched chunks, then w2/wh gated behind the ramp matmuls; the scalar
queue only carries the tiny bias vectors so the ACT engine stays free.
"""

import os
import numpy as np

B, IN_DIM, HIDDEN, H2 = 4096, 2048, 2048, 1024
V, R, L = 16, 32, 8
OUTC = V * (2 * R + 2 * L)          # 1280
NCORES = 8
BC = B // NCORES                    # 512 batch rows per core
KT1 = IN_DIM // 128                 # 16 k-tiles, layer 1
MT1 = HIDDEN // 128                 # 16 m-tiles, layer 1
KT2 = HIDDEN // 128                 # 16 k-tiles, layer 2
MT2 = H2 // 128                     # 8 m-tiles, layer 2
KTH = H2 // 128                     # 8 k-tiles, heads
BT = BC // 128                      # 4 batch tiles per core
CW = 320                            # head chunk width = 4 vehicles
NCH = OUTC // CW                    # 4 chunks
VC = CW // (2 * (R + L))            # 4 vehicles per chunk

_CACHE = {}
LAST_RESULTS = None                 # BassKernelResults from the last run


def _colperm():
    # device column g -> original output column. Within each 320-col chunk
    # (4 vehicles) the columns are grouped [8 rsu segs x R | 8 lay segs x L]
    # so the on-device softmax sees uniform 3D patterns.
    p = np.empty(OUTC, dtype=np.int64)
    g = 0
    for c in range(NCH):
        for s in range(2 * VC):
            v, h = 4 * c + s // 2, s % 2
            for j in range(R):
                p[g] = v * 2 * (R + L) + h * R + j
                g += 1
        for s in range(2 * VC):
            v, h = 4 * c + s // 2, s % 2
            for j in range(L):
                p[g] = v * 2 * (R + L) + 2 * R + h * L + j
                g += 1
    assert g == OUTC
    return p


_COLPERM = _colperm()


def _build():
    import bass_rust
    import concourse.bacc as bacc
    import concourse.mybir as mybir
    import concourse.tile as tile

    F32 = mybir.dt.float32
    BF16 = mybir.dt.bfloat16
    F8 = mybir.dt.float8e4
    DR = mybir.MatmulPerfMode.DoubleRow
    Relu = mybir.ActivationFunctionType.Relu
    Exp = mybir.ActivationFunctionType.Exp
    X = mybir.AxisListType.X

    nc = bacc.Bacc("TRN2", target_bir_lowering=False, debug=False,
                   num_devices=NCORES)

    xt = nc.dram_tensor("xt", [128, KT1, BC], F8, kind="ExternalInput")
    w1t = nc.dram_tensor("w1t", [128, MT1, KT1, 128], F8, kind="ExternalInput")
    b1c = nc.dram_tensor("b1c", [128, MT1], F32, kind="ExternalInput")
    w2t = nc.dram_tensor("w2t", [128, MT2, KT2, 128], F8, kind="ExternalInput")
    b2c = nc.dram_tensor("b2c", [128, MT2], F32, kind="ExternalInput")
    # kp = KTH//2 is a bias pad: row (p=0, sub=0) of h2 holds a constant 16,
    # the matching wht rows hold 1024*bh, everything else zero. Columns are
    # grouped [all rsu-head segments | all lay-head segments] so the softmax
    # reductions see uniform 3D access patterns; the host unpermutes.
    wht = nc.dram_tensor("wht", [128, KTH // 2 + 1, 2, OUTC], F8,
                         kind="ExternalInput")
    out = nc.dram_tensor("out", [BC, OUTC], BF16, kind="ExternalOutput")

    with tile.TileContext(nc) as tc:
        with (
            tc.tile_pool(name="const", bufs=1) as cp,
            tc.tile_pool(name="wpool", bufs=8) as wp,
            tc.tile_pool(name="sm", bufs=4) as sp,
            tc.tile_pool(name="ps", bufs=4, space="PSUM") as ps,
            tc.tile_pool(name="psh", bufs=4, space="PSUM") as psh,
        ):
            # PE warmup: dummy matmuls bridge the gap until the first w1/xt
            # tiles land so the HAM clock ramp overlaps the DMA-bound start.
            warm = cp.tile([128, 128], BF16, tag="warm")
            nc.gpsimd.memset(warm[:], 0.0)
            wacc = psh.tile([128, CW], F32, tag="hacc")
            for i in range(20):
                nc.tensor.matmul(wacc[:, 0:128], warm[:], warm[:],
                                 start=True, stop=True)

            xt_sb = cp.tile([128, KT1, BC], F8, tag="xt")
            h1_sb = cp.tile([128, MT1, BC], F8, tag="h1")
            # pre-tiled for DoubleRow: [kpair][bt|chunk][2][dim] slices are
            # contiguous [128, 2, d] blocks
            h2_sb = cp.tile([128, KTH // 2 + 1, BT, 2, 128], F8, tag="h2")
            wh_sb = cp.tile([128, KTH // 2 + 1, 2, OUTC], F8, tag="wh")
            w2_sb = cp.tile([128, MT2, KT2, 128], F8, tag="w2")
            b1_sb = cp.tile([128, MT1], F32, tag="b1")
            b2_sb = cp.tile([128, MT2], F32, tag="b2")
            # bias pad k-tiles of h2: all zero except partition-0/sub-0 = 16
            nc.vector.memset(h2_sb[:, KTH // 2], 0.0)
            nc.vector.memset(h2_sb[0:1, KTH // 2, :, 0, :], 16.0)

            # scalar queue, strict FIFO and few issues: b1, xt (small first
            # chunk so the ramp can start), b2, bh. w2/wh go on the gpsimd
            # SWDGE queue, gated behind L1 progress so their transfers can't
            # steal HBM bandwidth from the w1/xt streams at the start.
            # xt rides the gpsimd SWDGE queue in fine chunks matched to the
            # ramp's k-pair consumption; w2/wh follow on the same queue,
            # gated behind the ramp matmuls. The scalar queue only carries
            # the tiny bias vectors, keeping the ACT engine free.
            for k0, kn in [(0, 2), (2, 2), (4, 4), (8, 4), (12, 4)]:
                nc.gpsimd.dma_start(xt_sb[:, k0:k0 + kn, :],
                                    xt.ap()[:, k0:k0 + kn, :])
            nc.scalar.dma_start(b1_sb[:], b1c.ap())
            nc.scalar.dma_start(b2_sb[:], b2c.ap())

            # sync queue: w1 only. Ramp tiles m0..3 arrive as half tiles so
            # the k-outer ramp is fed at finer granularity; the rest are
            # m-tile pairs with 4KB descriptor runs.
            RM = 4                       # m-tiles in the k-outer ramp group
            rw1 = []
            for mi in range(RM):
                w1m = wp.tile([128, KT1, 128], F8, tag="w1m")
                rw1.append(w1m)
            for k0 in range(0, KT1, 8):
                for mi in range(RM):
                    nc.sync.dma_start(rw1[mi][:, k0:k0 + 8, :],
                                      w1t.ap()[:, mi, k0:k0 + 8, :])

            raccs = []
            for mi in range(RM):
                racc = ps.tile([128, 512], F32, tag="acc")
                raccs.append(racc)
            ramp_mm = None
            for k in range(0, KT1, 2):
                for mi in range(RM):
                    ramp_mm = nc.tensor.matmul(raccs[mi][:],
                                               rw1[mi][:, k:k + 2, :],
                                               xt_sb[:, k:k + 2, :],
                                               start=(k == 0),
                                               stop=(k == KT1 - 2),
                                               perf_mode=DR)
            for mi in range(RM):
                nc.scalar.activation(h1_sb[:, mi, :], raccs[mi][:],
                                     Relu, bias=b1_sb[:, mi:mi + 1],
                                     scale=1.0 / 512.0)

            # w2/wh prefetch on gpsimd, paced behind the ramp matmuls (by
            # then xt is fully resident and the w1 stream has headroom)
            for m0 in range(0, MT2, 4):
                w2d = nc.gpsimd.dma_start(w2_sb[:, m0:m0 + 4],
                                          w2t.ap()[:, m0:m0 + 4])
                bass_rust.add_dep_helper(w2d.ins, ramp_mm.ins, sync=True,
                                         reason="pace w2 prefetch")
            whd = nc.gpsimd.dma_start(wh_sb[:], wht.ap())
            bass_rust.add_dep_helper(whd.ins, ramp_mm.ins, sync=True,
                                     reason="pace wh prefetch")

            # Layer 1 remainder, m-tile pairs: h1[m] = relu(w1[:,m].T @ xt + b1)
            for m0 in range(RM, MT1, 2):
                w1m2 = wp.tile([128, 2, KT1, 128], F8, tag="w1m2")
                nc.sync.dma_start(w1m2[:], w1t.ap()[:, m0:m0 + 2])
                for j in range(2):
                    m = m0 + j
                    acc = ps.tile([128, 512], F32, tag="acc")
                    for k in range(0, KT1, 2):
                        nc.tensor.matmul(acc[:], w1m2[:, j, k:k + 2, :],
                                         xt_sb[:, k:k + 2, :],
                                         start=(k == 0), stop=(k == KT1 - 2),
                                         perf_mode=DR)
                    nc.scalar.activation(h1_sb[:, m, :], acc[:], Relu,
                                         bias=b1_sb[:, m:m + 1],
                                         scale=1.0 / 512.0)

            # Layer 2: h2[m] = relu(sum_k w2[k,m].T @ h1[k] + b2[m]),
            # stored x64 in fp8 for the DoubleRow head matmuls.
            for m in range(MT2):
                acc = ps.tile([128, 512], F32, tag="acc")
                for k in range(0, KT2, 2):
                    nc.tensor.matmul(acc[:], w2_sb[:, m, k:k + 2, :],
                                     h1_sb[:, k:k + 2, :],
                                     start=(k == 0), stop=(k == KT2 - 2),
                                     perf_mode=DR)
                nc.scalar.activation(h2_sb[:, m // 2, :, m % 2, :], acc[:],
                                     Relu, bias=b2_sb[:, m:m + 1],
                                     scale=1.0 / 64.0)

            # Heads: acc[b, c] = 16384*(h2[:, b].T @ wh[:, c] + bh[c]),
            # fp8 DoubleRow + a ones-row bf16 matmul for the bias. Softmax:
            # exp on ACT (scale folds the x16384 away), f32 reduces + recip
            # on DVE, divisor muls on GpSimd (split with DVE for the last
            # pair to shorten the drain), bf16 stores on the idle sync queue.
            # Per-chunk pipeline with a 4-deep PSUM ring: matmuls for chunk
            # c+3 can run while chunks c..c+2 drain through exp/reduce/mul.
            # Reciprocal is batched per chunk-pair to bound DVE overheads.
            for bt in range(BT):
                bsl = slice(bt * 128, (bt + 1) * 128)
                for c in range(NCH):
                    acc = psh.tile([128, CW], F32, tag="hacc")
                    for kp in range(KTH // 2 + 1):
                        nc.tensor.matmul(acc[:], h2_sb[:, kp, bt],
                                         wh_sb[:, kp, :,
                                               c * CW:(c + 1) * CW],
                                         start=(kp == 0),
                                         stop=(kp == KTH // 2),
                                         perf_mode=DR)

                    if c % 2 == 0:
                        et = sp.tile([128, 2 * CW], F32, tag="et")
                        sums = sp.tile([128, 8 * VC], F32, tag="sums")
                        rec = sp.tile([128, 8 * VC], F32, tag="rec")
                        o_sb = sp.tile([128, 2 * CW], BF16, tag="o")
                    half = (c % 2) * CW
                    nc.scalar.activation(et[:, half:half + CW], acc[:], Exp,
                                         scale=1.0 / 16384.0)

                    # grouped columns: chunk = [2*VC rsu segs x R | 2*VC lay
                    # segs x L] -- flat 3D views for the DVE
                    NRC = 2 * VC * R
                    rsu3 = et[:, half:half + NRC].rearrange(
                        "p (s c) -> p s c", s=2 * VC)
                    lay3 = et[:, half + NRC:half + CW].rearrange(
                        "p (s c) -> p s c", s=2 * VC)
                    sh = (c % 2) * 4 * VC
                    nc.vector.reduce_sum(
                        out=sums[:, sh:sh + 2 * VC].unsqueeze(2),
                        in_=rsu3, axis=X)
                    nc.vector.reduce_sum(
                        out=sums[:, sh + 2 * VC:sh + 4 * VC].unsqueeze(2),
                        in_=lay3, axis=X)
                    if c % 2 == 1:
                        nc.vector.reciprocal(rec[:, 0:8 * VC],
                                             sums[:, 0:8 * VC])
                    last = (bt == BT - 1 and c == NCH - 1)

                    # divisor muls trail one chunk behind the recip
                    if c % 2 == 1:
                        for cc in (c - 1, c):
                            h2_ = (cc % 2) * CW
                            sh2 = (cc % 2) * 4 * VC
                            orsu = o_sb[:, h2_:h2_ + NRC].rearrange(
                                "p (s c) -> p s c", s=2 * VC)
                            olay = o_sb[:, h2_ + NRC:h2_ + CW].rearrange(
                                "p (s c) -> p s c", s=2 * VC)
                            ersu = et[:, h2_:h2_ + NRC].rearrange(
                                "p (s c) -> p s c", s=2 * VC)
                            elay = et[:, h2_ + NRC:h2_ + CW].rearrange(
                                "p (s c) -> p s c", s=2 * VC)
                            r_r = rec[:, sh2:sh2 + 2 * VC]
                            r_l = rec[:, sh2 + 2 * VC:sh2 + 4 * VC]
                            nc.gpsimd.tensor_mul(
                                orsu, ersu,
                                r_r.unsqueeze(2).broadcast_to(
                                    [128, 2 * VC, R]))
                            nc.gpsimd.tensor_mul(
                                olay, elay,
                                r_l.unsqueeze(2).broadcast_to(
                                    [128, 2 * VC, L]))
                            if last:
                                nc.sync.dma_start(
                                    out.ap()[bsl, cc * CW:(cc + 1) * CW],
                                    o_sb[:, h2_:h2_ + CW])
                        if not last:
                            nc.sync.dma_start(
                                out.ap()[bsl, (c - 1) * CW:(c + 1) * CW],
                                o_sb[:, 0:2 * CW])

    nc.compile()
    return nc


def _prep_shared(w1, b1, w2, b2, w_rsu, b_rsu, w_lay, b_lay):
    import ml_dtypes
    f, bf = np.float32, ml_dtypes.bfloat16
    f8 = ml_dtypes.float8_e4m3
    # partition-major [128, MT1, KT1, 128]: paired-tile DMA slices match the
    # SBUF destination dim order and give 4KB-contiguous runs per partition
    w1t = np.ascontiguousarray(
        np.clip(w1 * 256.0, -240, 240).astype(f8)
        .reshape(KT1, 128, MT1, 128).transpose(1, 2, 0, 3))
    # partition-major [128, MT2, KT2, 128] so paired-tile DMA slices match
    # the SBUF destination's dimension order
    w2t = np.ascontiguousarray(
        np.clip(w2 * 256.0, -240, 240).astype(f8)
        .reshape(KT2, 128, MT2, 128).transpose(1, 2, 0, 3))
    b1c = np.ascontiguousarray(16.0 * b1.reshape(MT1, 128).T, dtype=f)
    b2c = np.ascontiguousarray(64.0 * b2.reshape(MT2, 128).T, dtype=f)

    wh = np.empty((H2, OUTC), dtype=f)
    bh = np.empty((OUTC,), dtype=f)
    for v in range(V):
        c = 2 * (R + L) * v
        wh[:, c:c + R] = w_rsu[2 * v]
        wh[:, c + R:c + 2 * R] = w_rsu[2 * v + 1]
        wh[:, c + 2 * R:c + 2 * R + L] = w_lay[2 * v]
        wh[:, c + 2 * R + L:c + 2 * (R + L)] = w_lay[2 * v + 1]
        bh[c:c + R] = b_rsu[2 * v]
        bh[c + R:c + 2 * R] = b_rsu[2 * v + 1]
        bh[c + 2 * R:c + 2 * R + L] = b_lay[2 * v]
        bh[c + 2 * R + L:c + 2 * (R + L)] = b_lay[2 * v + 1]
    # group columns [all rsu-head segments | all lay-head segments] so the
    # on-device softmax sees uniform 3D patterns; _COLPERM maps back
    whg = wh[:, _COLPERM]
    bhg = bh[_COLPERM]
    # [128p, kpair, sub, col]; the extra kpair is the bias pad: row
    # (p=0, sub=0) carries 1024*bh, pairing with h2's constant-16 row to
    # add 16384*bh to the logits.
    whp = np.clip(whg * 256.0, -240, 240).astype(f8) \
        .reshape(KTH // 2, 2, 128, OUTC).transpose(2, 0, 1, 3)
    wht = np.zeros((128, KTH // 2 + 1, 2, OUTC), dtype=f8)
    wht[:, :KTH // 2] = whp
    wht[0, KTH // 2, 0, :] = np.clip(1024.0 * bhg, -240, 240).astype(f8)
    return {"w1t": w1t, "b1c": b1c, "w2t": w2t, "b2c": b2c, "wht": wht}


def kernel(x, w1, b1, w2, b2, w_rsu, b_rsu, w_lay, b_lay):
    global LAST_RESULTS
    import ml_dtypes
    from concourse.bass_utils import run_bass_kernel_spmd

    if "nc" not in _CACHE:
        _CACHE["nc"] = _build()
    nc = _CACHE["nc"]

    shared = _prep_shared(np.asarray(w1, np.float32), np.asarray(b1, np.float32),
                          np.asarray(w2, np.float32), np.asarray(b2, np.float32),
                          np.asarray(w_rsu, np.float32), np.asarray(b_rsu, np.float32),
                          np.asarray(w_lay, np.float32), np.asarray(b_lay, np.float32))

    # x [B, IN] -> per-core xt [128, KT1, BC] with [p, k, n] = x[core*BC+n, k*128+p]
    # fp8 e4m3 with x*32 so small values clear the subnormal range; the
    # combined 32*256 scale comes out in the L1 relu (scale=1/512 with h1 x16)
    xt_full = np.clip(np.ascontiguousarray(np.asarray(x, np.float32).T) * 32.0,
                      -240, 240) \
        .astype(ml_dtypes.float8_e4m3).reshape(KT1, 128, B).transpose(1, 0, 2)
    in_maps = []
    for c in range(NCORES):
        m = dict(shared)
        m["xt"] = np.ascontiguousarray(xt_full[:, :, c * BC:(c + 1) * BC])
        in_maps.append(m)

    trace = os.environ.get("KERNEL_TRACE", "") == "1"
    LAST_RESULTS = run_bass_kernel_spmd(nc, in_maps, core_ids=list(range(NCORES)),
                                        trace=trace)
    dev = np.concatenate(
        [r["out"].astype(np.float32) for r in LAST_RESULTS.results], axis=0)
    full = np.empty_like(dev)
    full[:, _COLPERM] = dev          # undo the grouped column order
    return full
